# revision 1
# baseline (speedup 1.0000x reference)
"""Trainium2 Bass kernel for nn_DifferentiableLindblad.

Math: the reference Liouvillian decomposes as
    out[b] = DECAY + 1j * (X[b] @ G).reshape(16, 16)
where
    X[b] = [Omega[b], Delta+dd1+dph, Delta+dd2+dph, V_vdW[b]]   (4 scalars)
    G    = stack of 4 constant (16,16) generators kron(I,A) - kron(A,I),
           A in {H_drive, -N1, -N2, N_RR}, flattened to (4, 256)
    DECAY = constant real (16,16) decay superoperator.

Device work (data parallel over 8 NeuronCores, batch 65536 -> 8192/core):
per core a (8192, 4)-by-(4, 256) matmul producing the f32 imaginary part
(8 MiB/core written to HBM — the memory-bound term). The constant real
part and the f32->complex128 widening happen on host (pure broadcasting
of a constant; no per-element math).

Layout trick: each N=512 matmul (K=8, block-diag G) computes TWO
consecutive batch rows per PSUM partition, so PSUM rows map to contiguous
2 KiB runs of the (8192, 256) row-major output and output DMAs are large
and nearly contiguous.
"""

import numpy as np

B = 65536
NCORES = 8
BC = B // NCORES          # 8192 batch elements per core
NMM = BC // 256           # 32 matmuls per core (each covers 256 batch rows)
STAGES = 4                # output DMA groups per core
MM_PER_STAGE = NMM // STAGES

DIM = 4
SUP = 16
GAMMA = 1.0 / 88e-6


def _build_constants():
    """Rebuild the reference's constant operators in pure numpy (f64)."""
    g = np.array([1, 0], dtype=complex)
    r = np.array([0, 1], dtype=complex)
    s_gr = np.outer(g, r)
    s_rg = np.outer(r, g)
    n_r = np.outer(r, r)
    I2 = np.eye(2)
    s_gr1 = np.kron(s_gr, I2)
    s_rg1 = np.kron(s_rg, I2)
    n1 = np.kron(n_r, I2)
    s_gr2 = np.kron(I2, s_gr)
    s_rg2 = np.kron(I2, s_rg)
    n2 = np.kron(I2, n_r)
    H_drive = 0.5 * (s_rg1 + s_gr1 + s_rg2 + s_gr2)
    n_rr = n1 @ n2
    I4 = np.eye(DIM)
    decay = np.zeros((SUP, SUP), dtype=complex)
    for c in (np.sqrt(GAMMA) * s_gr1, np.sqrt(GAMMA) * s_gr2):
        cdc = c.conj().T @ c
        decay += np.kron(c, c.conj()) - 0.5 * (np.kron(cdc, I4) + np.kron(I4, cdc.T))

    def gen(A):
        return np.kron(I4, A) - np.kron(A, I4)

    G = np.stack(
        [
            gen(H_drive).real.reshape(SUP * SUP),
            gen(-n1).real.reshape(SUP * SUP),
            gen(-n2).real.reshape(SUP * SUP),
            gen(n_rr).real.reshape(SUP * SUP),
        ],
        axis=0,
    )  # (4, 256) f64
    return decay.real, G


DECAY_REAL, G_MAT = _build_constants()

# Block-diagonal rhs for the K=8 / N=512 matmul: rows 0:4 produce even
# batch rows (psum cols 0:256), rows 4:8 odd batch rows (cols 256:512).
G8 = np.zeros((8, 512), dtype=np.float32)
G8[0:4, 0:256] = G_MAT.astype(np.float32)
G8[4:8, 256:512] = G_MAT.astype(np.float32)

_CACHE = {}


def _build_module():
    """Build + compile the per-core Bass module (cached across calls)."""
    if "nc" in _CACHE:
        return _CACHE["nc"]

    import concourse.bacc as bacc
    import concourse.mybir as mybir
    import concourse.tile as tile

    f32 = mybir.dt.float32

    nc = bacc.Bacc("TRN2", target_bir_lowering=False, debug=False,
                   num_devices=NCORES)

    xt = nc.dram_tensor("xt", (8, NMM * 128), f32, kind="ExternalInput").ap()
    gmat = nc.dram_tensor("gmat", (8, 512), f32, kind="ExternalInput").ap()
    out = nc.dram_tensor("out", (BC, 256), f32, kind="ExternalOutput").ap()

    # DMA destination view: stage s covers batch rows [s*2048, (s+1)*2048).
    # Iteration order must match the SBUF stage tile [partition m][free f]
    # with f = jj*512 + (h*256 + c):  row = s*2048 + jj*256 + 2*m + h.
    out_v = out.rearrange("(s jj m h) c -> s m jj (h c)",
                          s=STAGES, jj=MM_PER_STAGE, m=128, h=2)

    with tile.TileContext(nc) as tc:
        with (
            tc.tile_pool(name="const", bufs=1) as cpool,
            tc.tile_pool(name="psum", bufs=8, space="PSUM") as ppool,
            tc.tile_pool(name="stage", bufs=2) as spool,
        ):
            xt_t = cpool.tile([8, NMM * 128], f32)
            nc.sync.dma_start(xt_t[:], xt)
            g_t = cpool.tile([8, 512], f32)
            nc.sync.dma_start(g_t[:], gmat)

            for s in range(STAGES):
                stage = spool.tile([128, MM_PER_STAGE * 512], f32)
                for jj in range(MM_PER_STAGE):
                    j = s * MM_PER_STAGE + jj
                    ps = ppool.tile([128, 512], f32)
                    nc.tensor.matmul(
                        ps[:],
                        lhsT=xt_t[:, j * 128:(j + 1) * 128],
                        rhs=g_t[:],
                        start=True,
                        stop=True,
                    )
                    dst = stage[:, jj * 512:(jj + 1) * 512]
                    if jj % 2 == 0:
                        nc.vector.tensor_copy(dst, ps[:])
                    else:
                        nc.scalar.copy(dst, ps[:])
                nc.sync.dma_start(
                    out_v[s], stage[:].rearrange("m (jj f) -> m jj f",
                                                 jj=MM_PER_STAGE)
                )

    nc.compile()
    _CACHE["nc"] = nc
    return nc


def _pack_xt(om, d1, d2, v):
    """Pack per-core X^T (8, NMM*128): row r, col j*128+m = X component of
    batch element j*256 + 2m (+1 for rows 4:8)."""
    xt = np.empty((8, NMM * 128), dtype=np.float32)
    for r, arr in enumerate((om, d1, d2, v)):
        a = arr.reshape(NMM, 128, 2)
        xt[r] = a[:, :, 0].reshape(-1)
        xt[r + 4] = a[:, :, 1].reshape(-1)
    return xt


def kernel(Omega, Delta, delta_doppler_1, delta_doppler_2, delta_phase,
           V_vdW):
    from concourse.bass_utils import run_bass_kernel_spmd

    nc = _build_module()

    Omega = np.ascontiguousarray(Omega, dtype=np.float32)
    V_vdW = np.ascontiguousarray(V_vdW, dtype=np.float32)
    d1 = (Delta + delta_doppler_1 + delta_phase).astype(np.float32)
    d2 = (Delta + delta_doppler_2 + delta_phase).astype(np.float32)

    in_maps = []
    for c in range(NCORES):
        sl = slice(c * BC, (c + 1) * BC)
        in_maps.append({
            "xt": _pack_xt(Omega[sl], d1[sl], d2[sl], V_vdW[sl]),
            "gmat": G8,
        })

    res = run_bass_kernel_spmd(nc, in_maps, core_ids=list(range(NCORES)))

    imag = np.concatenate([res.results[c]["out"] for c in range(NCORES)],
                          axis=0)

    out = np.empty((B, SUP, SUP), dtype=np.complex128)
    out.real[...] = DECAY_REAL[None]
    out.imag[...] = imag.reshape(B, SUP, SUP)
    return out


# revision 6
# speedup vs baseline: 1.5909x; 1.5909x over previous
"""Trainium2 Bass kernel for nn_DifferentiableLindblad.

Math: the reference Liouvillian decomposes as
    out[b] = DECAY + 1j * (X[b] @ G).reshape(16, 16)
where
    X[b] = [Omega[b], Delta+dd1+dph, Delta+dd2+dph, V_vdW[b]]   (4 scalars)
    G    = stack of 4 constant (16,16) generators kron(I,A) - kron(A,I),
           A in {H_drive, -N1, -N2, N_RR}, flattened to (4, 256)
    DECAY = constant real (16,16) decay superoperator.

Device work (data parallel over 8 NeuronCores, batch 65536 -> 8192/core):
per core a (8192, 4)-by-(4, 256) matmul producing the f32 imaginary part
(8 MiB/core written to HBM — the memory-bound term). The constant real
part and the f32->complex128 widening happen on host (pure broadcasting
of a constant; no per-element math).

Layout trick: each N=512 matmul (K=8, block-diag G) computes TWO
consecutive batch rows per PSUM partition, so PSUM rows map to contiguous
2 KiB runs of the (8192, 256) row-major output and output DMAs are large
and nearly contiguous.
"""

import numpy as np

B = 65536
NCORES = 8
BC = B // NCORES          # 8192 batch elements per core
NMM = BC // 256           # 32 matmuls per core (each covers 256 batch rows)
STAGES = 4                # output DMA groups per core
MM_PER_STAGE = NMM // STAGES

DIM = 4
SUP = 16
GAMMA = 1.0 / 88e-6


def _build_constants():
    """Rebuild the reference's constant operators in pure numpy (f64)."""
    g = np.array([1, 0], dtype=complex)
    r = np.array([0, 1], dtype=complex)
    s_gr = np.outer(g, r)
    s_rg = np.outer(r, g)
    n_r = np.outer(r, r)
    I2 = np.eye(2)
    s_gr1 = np.kron(s_gr, I2)
    s_rg1 = np.kron(s_rg, I2)
    n1 = np.kron(n_r, I2)
    s_gr2 = np.kron(I2, s_gr)
    s_rg2 = np.kron(I2, s_rg)
    n2 = np.kron(I2, n_r)
    H_drive = 0.5 * (s_rg1 + s_gr1 + s_rg2 + s_gr2)
    n_rr = n1 @ n2
    I4 = np.eye(DIM)
    decay = np.zeros((SUP, SUP), dtype=complex)
    for c in (np.sqrt(GAMMA) * s_gr1, np.sqrt(GAMMA) * s_gr2):
        cdc = c.conj().T @ c
        decay += np.kron(c, c.conj()) - 0.5 * (np.kron(cdc, I4) + np.kron(I4, cdc.T))

    def gen(A):
        return np.kron(I4, A) - np.kron(A, I4)

    G = np.stack(
        [
            gen(H_drive).real.reshape(SUP * SUP),
            gen(-n1).real.reshape(SUP * SUP),
            gen(-n2).real.reshape(SUP * SUP),
            gen(n_rr).real.reshape(SUP * SUP),
        ],
        axis=0,
    )  # (4, 256) f64
    return decay.real, G


DECAY_REAL, G_MAT = _build_constants()

# Block-diagonal rhs for the K=8 / N=512 matmul: rows 0:4 produce even
# batch rows (psum cols 0:256), rows 4:8 odd batch rows (cols 256:512).
# G entries are {0, ±0.5, ±1} — exact in fp16. X is fed as a 2-term fp16
# split (hi + residual): fp16 streams through the PE at full rate where
# fp32 runs at 1/4 rate, and two accumulating fp16 matmuls reproduce the
# fp32 product to ~2^-22, far inside the fp32 envelope of this output.
G8 = np.zeros((8, 512), dtype=np.float16)
G8[0:4, 0:256] = G_MAT.astype(np.float16)
G8[4:8, 256:512] = G_MAT.astype(np.float16)

_CACHE = {}


def _build_module():
    """Build + compile the per-core Bass module (cached across calls)."""
    if "nc" in _CACHE:
        return _CACHE["nc"]

    import concourse.bacc as bacc
    import concourse.mybir as mybir
    import concourse.tile as tile

    f32 = mybir.dt.float32
    f16 = mybir.dt.float16

    nc = bacc.Bacc("TRN2", target_bir_lowering=False, debug=False,
                   num_devices=NCORES)

    xt_hi = nc.dram_tensor("xt_hi", (8, NMM * 128), f16,
                           kind="ExternalInput").ap()
    xt_lo = nc.dram_tensor("xt_lo", (8, NMM * 128), f16,
                           kind="ExternalInput").ap()
    gmat = nc.dram_tensor("gmat", (8, 512), f16, kind="ExternalInput").ap()
    out = nc.dram_tensor("out", (BC, 256), f32, kind="ExternalOutput").ap()

    # DMA destination view: stage s covers batch rows [s*2048, (s+1)*2048).
    # Iteration order must match the SBUF stage tile [partition m][free f]
    # with f = jj*512 + (h*256 + c):  row = s*2048 + jj*256 + 2*m + h.
    out_v = out.rearrange("(s jj m h) c -> s m jj (h c)",
                          s=STAGES, jj=MM_PER_STAGE, m=128, h=2)

    with tile.TileContext(nc) as tc:
        with (
            tc.tile_pool(name="const", bufs=1) as cpool,
            tc.tile_pool(name="psum", bufs=8, space="PSUM") as ppool,
            tc.tile_pool(name="stage", bufs=2) as spool,
        ):
            xh_t = cpool.tile([8, NMM * 128], f16)
            nc.sync.dma_start(xh_t[:], xt_hi)
            xl_t = cpool.tile([8, NMM * 128], f16)
            nc.sync.dma_start(xl_t[:], xt_lo)
            g_t = cpool.tile([8, 512], f16)
            nc.sync.dma_start(g_t[:], gmat)

            for s in range(STAGES):
                stage = spool.tile([128, MM_PER_STAGE * 512], f32)
                for jj in range(MM_PER_STAGE):
                    j = s * MM_PER_STAGE + jj
                    ps = ppool.tile([128, 512], f32)
                    nc.tensor.matmul(
                        ps[:],
                        lhsT=xh_t[:, j * 128:(j + 1) * 128],
                        rhs=g_t[:],
                        start=True,
                        stop=False,
                    )
                    nc.tensor.matmul(
                        ps[:],
                        lhsT=xl_t[:, j * 128:(j + 1) * 128],
                        rhs=g_t[:],
                        start=False,
                        stop=True,
                    )
                    dst = stage[:, jj * 512:(jj + 1) * 512]
                    if jj % 2 == 0:
                        nc.vector.tensor_copy(dst, ps[:])
                    else:
                        nc.scalar.copy(dst, ps[:])
                nc.sync.dma_start(
                    out_v[s], stage[:].rearrange("m (jj f) -> m jj f",
                                                 jj=MM_PER_STAGE)
                )

    nc.compile()
    _CACHE["nc"] = nc
    return nc


def _pack_xt(om, d1, d2, v):
    """Pack per-core X^T (8, NMM*128) and split into fp16 hi + residual:
    row r, col j*128+m = X component of batch element j*256 + 2m
    (+1 for rows 4:8)."""
    xt = np.empty((8, NMM * 128), dtype=np.float32)
    for r, arr in enumerate((om, d1, d2, v)):
        a = arr.reshape(NMM, 128, 2)
        xt[r] = a[:, :, 0].reshape(-1)
        xt[r + 4] = a[:, :, 1].reshape(-1)
    hi = xt.astype(np.float16)
    lo = (xt - hi.astype(np.float32)).astype(np.float16)
    return hi, lo


def kernel(Omega, Delta, delta_doppler_1, delta_doppler_2, delta_phase,
           V_vdW):
    from concourse.bass_utils import run_bass_kernel_spmd

    nc = _build_module()

    Omega = np.ascontiguousarray(Omega, dtype=np.float32)
    V_vdW = np.ascontiguousarray(V_vdW, dtype=np.float32)
    d1 = (Delta + delta_doppler_1 + delta_phase).astype(np.float32)
    d2 = (Delta + delta_doppler_2 + delta_phase).astype(np.float32)

    in_maps = []
    for c in range(NCORES):
        sl = slice(c * BC, (c + 1) * BC)
        hi, lo = _pack_xt(Omega[sl], d1[sl], d2[sl], V_vdW[sl])
        in_maps.append({"xt_hi": hi, "xt_lo": lo, "gmat": G8})

    res = run_bass_kernel_spmd(nc, in_maps, core_ids=list(range(NCORES)))

    imag = np.concatenate([res.results[c]["out"] for c in range(NCORES)],
                          axis=0)

    out = np.empty((B, SUP, SUP), dtype=np.complex128)
    out.real[...] = DECAY_REAL[None]
    out.imag[...] = imag.reshape(B, SUP, SUP)
    return out


# revision 11
# speedup vs baseline: 1.7911x; 1.1258x over previous
"""Trainium2 Bass kernel for nn_DifferentiableLindblad.

Math: the reference Liouvillian decomposes as
    out[b] = DECAY + 1j * (X[b] @ G).reshape(16, 16)
where
    X[b] = [Omega[b], Delta+dd1+dph, Delta+dd2+dph, V_vdW[b]]   (4 scalars)
    G    = stack of 4 constant (16,16) generators kron(I,A) - kron(A,I),
           A in {H_drive, -N1, -N2, N_RR}, flattened to (4, 256)
    DECAY = constant real (16,16) decay superoperator.

Device work (data parallel over 8 NeuronCores, batch 65536 -> 8192/core):
per core a (8192, 4)-by-(4, 256) matmul producing the f32 imaginary part
(8 MiB/core written to HBM — the memory-bound term). The constant real
part and the f32->complex128 widening happen on host (pure broadcasting
of a constant; no per-element math).

Layout trick: each N=512 matmul (K=8, block-diag G) computes TWO
consecutive batch rows per PSUM partition, so PSUM rows map to contiguous
2 KiB runs of the (8192, 256) row-major output and output DMAs are large
and nearly contiguous.
"""

import numpy as np

B = 65536
NCORES = 8
BC = B // NCORES          # 8192 batch elements per core
NMM = BC // 256           # 32 matmuls per core (each covers 256 batch rows)
STAGES = 4                # output DMA groups per core
MM_PER_STAGE = NMM // STAGES

DIM = 4
SUP = 16
GAMMA = 1.0 / 88e-6


def _build_constants():
    """Rebuild the reference's constant operators in pure numpy (f64)."""
    g = np.array([1, 0], dtype=complex)
    r = np.array([0, 1], dtype=complex)
    s_gr = np.outer(g, r)
    s_rg = np.outer(r, g)
    n_r = np.outer(r, r)
    I2 = np.eye(2)
    s_gr1 = np.kron(s_gr, I2)
    s_rg1 = np.kron(s_rg, I2)
    n1 = np.kron(n_r, I2)
    s_gr2 = np.kron(I2, s_gr)
    s_rg2 = np.kron(I2, s_rg)
    n2 = np.kron(I2, n_r)
    H_drive = 0.5 * (s_rg1 + s_gr1 + s_rg2 + s_gr2)
    n_rr = n1 @ n2
    I4 = np.eye(DIM)
    decay = np.zeros((SUP, SUP), dtype=complex)
    for c in (np.sqrt(GAMMA) * s_gr1, np.sqrt(GAMMA) * s_gr2):
        cdc = c.conj().T @ c
        decay += np.kron(c, c.conj()) - 0.5 * (np.kron(cdc, I4) + np.kron(I4, cdc.T))

    def gen(A):
        return np.kron(I4, A) - np.kron(A, I4)

    G = np.stack(
        [
            gen(H_drive).real.reshape(SUP * SUP),
            gen(-n1).real.reshape(SUP * SUP),
            gen(-n2).real.reshape(SUP * SUP),
            gen(n_rr).real.reshape(SUP * SUP),
        ],
        axis=0,
    )  # (4, 256) f64
    return decay.real, G


DECAY_REAL, G_MAT = _build_constants()

# Block-diagonal rhs for the N=512 matmul: within each 8-row block, rows
# 0:4 produce even batch rows (psum cols 0:256), rows 4:8 odd batch rows
# (cols 256:512). G entries are {0, ±0.5, ±1} — exact in bf16. X is fed
# as a 3-term bf16 split (hi + mid + lo = exact fp32) STACKED ALONG K
# (K=24), so a single full-rate bf16 matmul contracts all three terms:
# fp32 would stream at 1/4 rate, and sequential accumulation passes would
# triple the streamed columns. The fp32 PSUM contraction restores the
# exact product.
import ml_dtypes

_G8 = np.zeros((8, 512), dtype=ml_dtypes.bfloat16)
_G8[0:4, 0:256] = G_MAT.astype(ml_dtypes.bfloat16)
_G8[4:8, 256:512] = G_MAT.astype(ml_dtypes.bfloat16)
G24 = np.vstack([_G8, _G8, _G8])  # (24, 512) bf16

_CACHE = {}


def _build_module():
    """Build + compile the per-core Bass module (cached across calls)."""
    if "nc" in _CACHE:
        return _CACHE["nc"]

    import concourse.bacc as bacc
    import concourse.mybir as mybir
    import concourse.tile as tile

    f32 = mybir.dt.float32
    bf16 = mybir.dt.bfloat16

    nc = bacc.Bacc("TRN2", target_bir_lowering=False, debug=False,
                   num_devices=NCORES)

    xt = nc.dram_tensor("xt", (24, NMM * 128), bf16,
                        kind="ExternalInput").ap()
    gmat = nc.dram_tensor("gmat", (24, 512), bf16, kind="ExternalInput").ap()
    out = nc.dram_tensor("out", (BC, 256), f32, kind="ExternalOutput").ap()

    # DMA destination view: stage s covers batch rows [s*2048, (s+1)*2048).
    # Iteration order must match the SBUF stage tile [partition m][free f]
    # with f = jj*512 + (h*256 + c):  row = s*2048 + jj*256 + 2*m + h.
    out_v = out.rearrange("(s jj m h) c -> s m jj (h c)",
                          s=STAGES, jj=MM_PER_STAGE, m=128, h=2)

    with tile.TileContext(nc) as tc:
        with (
            tc.tile_pool(name="const", bufs=1) as cpool,
            tc.tile_pool(name="psum", bufs=8, space="PSUM") as ppool,
            tc.tile_pool(name="stage", bufs=2) as spool,
        ):
            xt_t = cpool.tile([24, NMM * 128], bf16)
            nc.sync.dma_start(xt_t[:], xt)
            g_t = cpool.tile([24, 512], bf16)
            nc.sync.dma_start(g_t[:], gmat)

            for s in range(STAGES):
                stage = spool.tile([128, MM_PER_STAGE * 512], f32)
                for jj in range(MM_PER_STAGE):
                    j = s * MM_PER_STAGE + jj
                    ps = ppool.tile([128, 512], f32)
                    nc.tensor.matmul(
                        ps[:],
                        lhsT=xt_t[:, j * 128:(j + 1) * 128],
                        rhs=g_t[:],
                        start=True,
                        stop=True,
                    )
                    dst = stage[:, jj * 512:(jj + 1) * 512]
                    if jj % 2 == 0:
                        nc.vector.tensor_copy(dst, ps[:])
                    else:
                        nc.scalar.copy(dst, ps[:])
                nc.sync.dma_start(
                    out_v[s], stage[:].rearrange("m (jj f) -> m jj f",
                                                 jj=MM_PER_STAGE)
                )

    nc.compile()
    _CACHE["nc"] = nc
    return nc


def _pack_xt(om, d1, d2, v):
    """Pack per-core X^T and 3-term bf16 split (exact fp32) stacked on K:
    rows 0:8 = hi, 8:16 = mid, 16:24 = lo; within each 8-block, row r /
    col j*128+m = X component of batch element j*256 + 2m (+1 for rows
    4:8 of the block)."""
    xt = np.empty((8, NMM * 128), dtype=np.float32)
    for r, arr in enumerate((om, d1, d2, v)):
        a = arr.reshape(NMM, 128, 2)
        xt[r] = a[:, :, 0].reshape(-1)
        xt[r + 4] = a[:, :, 1].reshape(-1)
    bf = ml_dtypes.bfloat16
    hi = xt.astype(bf)
    r1 = xt - hi.astype(np.float32)
    mid = r1.astype(bf)
    lo = (r1 - mid.astype(np.float32)).astype(bf)
    return np.vstack([hi, mid, lo])  # (24, NMM*128) bf16


def kernel(Omega, Delta, delta_doppler_1, delta_doppler_2, delta_phase,
           V_vdW):
    from concourse.bass_utils import run_bass_kernel_spmd

    nc = _build_module()

    Omega = np.ascontiguousarray(Omega, dtype=np.float32)
    V_vdW = np.ascontiguousarray(V_vdW, dtype=np.float32)
    d1 = (Delta + delta_doppler_1 + delta_phase).astype(np.float32)
    d2 = (Delta + delta_doppler_2 + delta_phase).astype(np.float32)

    in_maps = []
    for c in range(NCORES):
        sl = slice(c * BC, (c + 1) * BC)
        in_maps.append({
            "xt": _pack_xt(Omega[sl], d1[sl], d2[sl], V_vdW[sl]),
            "gmat": G24,
        })

    res = run_bass_kernel_spmd(nc, in_maps, core_ids=list(range(NCORES)))

    imag = np.concatenate([res.results[c]["out"] for c in range(NCORES)],
                          axis=0)

    out = np.empty((B, SUP, SUP), dtype=np.complex128)
    out.real[...] = DECAY_REAL[None]
    out.imag[...] = imag.reshape(B, SUP, SUP)
    return out


# revision 12
# speedup vs baseline: 1.9562x; 1.0922x over previous
"""Trainium2 Bass kernel for nn_DifferentiableLindblad.

Math: the reference Liouvillian decomposes as
    out[b] = DECAY + 1j * (X[b] @ G).reshape(16, 16)
where
    X[b] = [Omega[b], Delta+dd1+dph, Delta+dd2+dph, V_vdW[b]]   (4 scalars)
    G    = stack of 4 constant (16,16) generators kron(I,A) - kron(A,I),
           A in {H_drive, -N1, -N2, N_RR}, flattened to (4, 256)
    DECAY = constant real (16,16) decay superoperator.

Only 76 of G's 256 columns are nonzero, and the real part is a constant,
so the only batch-dependent data is imag[:, nz] = X @ G[:, nz].

Device work (data parallel over 8 NeuronCores, batch 65536 -> 8192/core):
one transposed matmul chain per core producing out_T (76, 8192) f32 =
G_nz^T @ X^T. G_nz (stationary operand) is exact in bf16; X (moving
operand) is fed as a 3-term bf16 split (hi+mid+lo = exact fp32) stacked
along K (K=12), because bf16 streams through the PE at full rate while
fp32 streams at 1/4 rate. The fp32 PSUM contraction restores the exact
fp32 product. The host scatters the 76 columns into the zero imag plane
and adds the constant real part (pure broadcasting, no per-element math).
"""

import numpy as np
import ml_dtypes

B = 65536
NCORES = 8
BC = B // NCORES          # 8192 batch elements per core
NMM = BC // 512           # 16 matmuls per core (512 batch each)
STAGES = 4                # output DMA groups per core
MM_PER_STAGE = NMM // STAGES

DIM = 4
SUP = 16
GAMMA = 1.0 / 88e-6


def _build_constants():
    """Rebuild the reference's constant operators in pure numpy (f64)."""
    g = np.array([1, 0], dtype=complex)
    r = np.array([0, 1], dtype=complex)
    s_gr = np.outer(g, r)
    s_rg = np.outer(r, g)
    n_r = np.outer(r, r)
    I2 = np.eye(2)
    s_gr1 = np.kron(s_gr, I2)
    s_rg1 = np.kron(s_rg, I2)
    n1 = np.kron(n_r, I2)
    s_gr2 = np.kron(I2, s_gr)
    s_rg2 = np.kron(I2, s_rg)
    n2 = np.kron(I2, n_r)
    H_drive = 0.5 * (s_rg1 + s_gr1 + s_rg2 + s_gr2)
    n_rr = n1 @ n2
    I4 = np.eye(DIM)
    decay = np.zeros((SUP, SUP), dtype=complex)
    for c in (np.sqrt(GAMMA) * s_gr1, np.sqrt(GAMMA) * s_gr2):
        cdc = c.conj().T @ c
        decay += np.kron(c, c.conj()) - 0.5 * (np.kron(cdc, I4) + np.kron(I4, cdc.T))

    def gen(A):
        return np.kron(I4, A) - np.kron(A, I4)

    G = np.stack(
        [
            gen(H_drive).real.reshape(SUP * SUP),
            gen(-n1).real.reshape(SUP * SUP),
            gen(-n2).real.reshape(SUP * SUP),
            gen(n_rr).real.reshape(SUP * SUP),
        ],
        axis=0,
    )  # (4, 256) f64
    return decay.real, G


DECAY_REAL, G_MAT = _build_constants()

# Nonzero columns of G (76 of 256) — the only batch-dependent outputs.
NZ_COLS = np.flatnonzero(np.abs(G_MAT).sum(axis=0) != 0)
NNZ = len(NZ_COLS)  # 76

# Stationary operand: (12, NNZ) bf16 = 3 stacked copies of G_nz, matching
# the 3-term [hi; mid; lo] K-split of X. Entries are {0, ±0.5, ±1}: exact.
_Gnz = G_MAT[:, NZ_COLS].astype(ml_dtypes.bfloat16)
G12 = np.vstack([_Gnz, _Gnz, _Gnz])  # (12, 76)

_CACHE = {}


def _build_module():
    """Build + compile the per-core Bass module (cached across calls)."""
    if "nc" in _CACHE:
        return _CACHE["nc"]

    import concourse.bacc as bacc
    import concourse.mybir as mybir
    import concourse.tile as tile

    f32 = mybir.dt.float32
    bf16 = mybir.dt.bfloat16

    nc = bacc.Bacc("TRN2", target_bir_lowering=False, debug=False,
                   num_devices=NCORES)

    xt = nc.dram_tensor("xt", (12, BC), bf16, kind="ExternalInput").ap()
    gmat = nc.dram_tensor("gmat", (12, NNZ), bf16, kind="ExternalInput").ap()
    out = nc.dram_tensor("out", (NNZ, BC), f32, kind="ExternalOutput").ap()

    with tile.TileContext(nc) as tc:
        with (
            tc.tile_pool(name="const", bufs=1) as cpool,
            tc.tile_pool(name="psum", bufs=8, space="PSUM") as ppool,
            tc.tile_pool(name="stage", bufs=2) as spool,
        ):
            g_t = cpool.tile([12, NNZ], bf16)
            nc.sync.dma_start(g_t[:], gmat)
            xt_t = cpool.tile([12, BC], bf16)
            # chunked load so the first matmuls start early
            for s in range(STAGES):
                w = BC // STAGES
                nc.sync.dma_start(xt_t[:, s * w:(s + 1) * w],
                                  xt[:, s * w:(s + 1) * w])

            for s in range(STAGES):
                stage = spool.tile([NNZ, MM_PER_STAGE * 512], f32)
                for jj in range(MM_PER_STAGE):
                    j = s * MM_PER_STAGE + jj
                    ps = ppool.tile([NNZ, 512], f32)
                    nc.tensor.matmul(
                        ps[:],
                        lhsT=g_t[:],
                        rhs=xt_t[:, j * 512:(j + 1) * 512],
                        start=True,
                        stop=True,
                    )
                    dst = stage[:, jj * 512:(jj + 1) * 512]
                    if jj % 2 == 0:
                        nc.vector.tensor_copy(dst, ps[:])
                    else:
                        nc.scalar.copy(dst, ps[:])
                w = MM_PER_STAGE * 512
                nc.sync.dma_start(out[:, s * w:(s + 1) * w], stage[:])

    nc.compile()
    _CACHE["nc"] = nc
    return nc


def _pack_xt(om, d1, d2, v):
    """Per-core X^T (12, BC) bf16: rows [hi(4); mid(4); lo(4)] of the
    exact 3-term bf16 split of [Omega, d1, d2, V], batch along columns."""
    xt = np.stack([om, d1, d2, v], axis=0)  # (4, BC) f32
    bf = ml_dtypes.bfloat16
    hi = xt.astype(bf)
    r1 = xt - hi.astype(np.float32)
    mid = r1.astype(bf)
    lo = (r1 - mid.astype(np.float32)).astype(bf)
    return np.vstack([hi, mid, lo])  # (12, BC) bf16


def kernel(Omega, Delta, delta_doppler_1, delta_doppler_2, delta_phase,
           V_vdW):
    from concourse.bass_utils import run_bass_kernel_spmd

    nc = _build_module()

    Omega = np.ascontiguousarray(Omega, dtype=np.float32)
    V_vdW = np.ascontiguousarray(V_vdW, dtype=np.float32)
    d1 = (Delta + delta_doppler_1 + delta_phase).astype(np.float32)
    d2 = (Delta + delta_doppler_2 + delta_phase).astype(np.float32)

    in_maps = []
    for c in range(NCORES):
        sl = slice(c * BC, (c + 1) * BC)
        in_maps.append({
            "xt": _pack_xt(Omega[sl], d1[sl], d2[sl], V_vdW[sl]),
            "gmat": G12,
        })

    res = run_bass_kernel_spmd(nc, in_maps, core_ids=list(range(NCORES)))

    out = np.zeros((B, SUP * SUP), dtype=np.complex128)
    out.real[...] = DECAY_REAL.reshape(1, SUP * SUP)
    for c in range(NCORES):
        sl = slice(c * BC, (c + 1) * BC)
        out[sl, NZ_COLS] += 1j * res.results[c]["out"].T.astype(np.float64)
    return out.reshape(B, SUP, SUP)


# revision 13
# speedup vs baseline: 2.7978x; 1.4302x over previous
"""Trainium2 Bass kernel for nn_DifferentiableLindblad.

Math: the reference Liouvillian decomposes as
    out[b] = DECAY + 1j * (X[b] @ G).reshape(16, 16)
where
    X[b] = [Omega[b], Delta+dd1+dph, Delta+dd2+dph, V_vdW[b]]   (4 scalars)
    G    = stack of 4 constant (16,16) generators kron(I,A) - kron(A,I),
           A in {H_drive, -N1, -N2, N_RR}, flattened to (4, 256)
    DECAY = constant real (16,16) decay superoperator.

Only 76 of G's 256 columns are nonzero, and the real part is a constant,
so the only batch-dependent data is imag[:, nz] = X @ G[:, nz].

Device work (data parallel over 8 NeuronCores, batch 65536 -> 8192/core):
one transposed matmul chain per core producing out_T (76, 8192) f32 =
G_nz^T @ X^T. G_nz (stationary operand) is exact in bf16; X (moving
operand) is fed as a 3-term bf16 split (hi+mid+lo = exact fp32) stacked
along K (K=12), because bf16 streams through the PE at full rate while
fp32 streams at 1/4 rate. The fp32 PSUM contraction restores the exact
fp32 product. The host scatters the 76 columns into the zero imag plane
and adds the constant real part (pure broadcasting, no per-element math).
"""

import numpy as np
import ml_dtypes

B = 65536
NCORES = 8
BC = B // NCORES          # 8192 batch elements per core
NMM = BC // 512           # 16 matmuls per core (512 batch each)
STAGES = 4                # output DMA groups per core
MM_PER_STAGE = NMM // STAGES

DIM = 4
SUP = 16
GAMMA = 1.0 / 88e-6


def _build_constants():
    """Rebuild the reference's constant operators in pure numpy (f64)."""
    g = np.array([1, 0], dtype=complex)
    r = np.array([0, 1], dtype=complex)
    s_gr = np.outer(g, r)
    s_rg = np.outer(r, g)
    n_r = np.outer(r, r)
    I2 = np.eye(2)
    s_gr1 = np.kron(s_gr, I2)
    s_rg1 = np.kron(s_rg, I2)
    n1 = np.kron(n_r, I2)
    s_gr2 = np.kron(I2, s_gr)
    s_rg2 = np.kron(I2, s_rg)
    n2 = np.kron(I2, n_r)
    H_drive = 0.5 * (s_rg1 + s_gr1 + s_rg2 + s_gr2)
    n_rr = n1 @ n2
    I4 = np.eye(DIM)
    decay = np.zeros((SUP, SUP), dtype=complex)
    for c in (np.sqrt(GAMMA) * s_gr1, np.sqrt(GAMMA) * s_gr2):
        cdc = c.conj().T @ c
        decay += np.kron(c, c.conj()) - 0.5 * (np.kron(cdc, I4) + np.kron(I4, cdc.T))

    def gen(A):
        return np.kron(I4, A) - np.kron(A, I4)

    G = np.stack(
        [
            gen(H_drive).real.reshape(SUP * SUP),
            gen(-n1).real.reshape(SUP * SUP),
            gen(-n2).real.reshape(SUP * SUP),
            gen(n_rr).real.reshape(SUP * SUP),
        ],
        axis=0,
    )  # (4, 256) f64
    return decay.real, G


DECAY_REAL, G_MAT = _build_constants()

# Nonzero columns of G (76 of 256) — the only batch-dependent outputs.
# Padded to 128 with zero columns: the output DMA fans out across SDMA
# engines by partition, and a 128-partition source uses all 16 engines
# (a 76-partition source measured only 4 engines / ~1/4 bandwidth).
_nz = np.flatnonzero(np.abs(G_MAT).sum(axis=0) != 0)
_pad = np.setdiff1d(np.arange(SUP * SUP), _nz)[:128 - len(_nz)]
NZ_COLS = np.concatenate([_nz, _pad])
NNZ = len(NZ_COLS)  # 128

# Stationary operand: (12, NNZ) bf16 = 3 stacked copies of G_nz, matching
# the 3-term [hi; mid; lo] K-split of X. Entries are {0, ±0.5, ±1}: exact.
_Gnz = G_MAT[:, NZ_COLS].astype(ml_dtypes.bfloat16)
G12 = np.vstack([_Gnz, _Gnz, _Gnz])  # (12, 128)

_CACHE = {}


def _build_module():
    """Build + compile the per-core Bass module (cached across calls)."""
    if "nc" in _CACHE:
        return _CACHE["nc"]

    import concourse.bacc as bacc
    import concourse.mybir as mybir
    import concourse.tile as tile

    f32 = mybir.dt.float32
    bf16 = mybir.dt.bfloat16

    nc = bacc.Bacc("TRN2", target_bir_lowering=False, debug=False,
                   num_devices=NCORES)

    xt = nc.dram_tensor("xt", (12, BC), bf16, kind="ExternalInput").ap()
    gmat = nc.dram_tensor("gmat", (12, NNZ), bf16, kind="ExternalInput").ap()
    out = nc.dram_tensor("out", (NNZ, BC), f32, kind="ExternalOutput").ap()

    with tile.TileContext(nc) as tc:
        with (
            tc.tile_pool(name="const", bufs=1) as cpool,
            tc.tile_pool(name="psum", bufs=8, space="PSUM") as ppool,
            tc.tile_pool(name="stage", bufs=2) as spool,
        ):
            g_t = cpool.tile([12, NNZ], bf16)
            nc.sync.dma_start(g_t[:], gmat)
            xt_t = cpool.tile([12, BC], bf16)
            # chunked load so the first matmuls start early
            for s in range(STAGES):
                w = BC // STAGES
                nc.sync.dma_start(xt_t[:, s * w:(s + 1) * w],
                                  xt[:, s * w:(s + 1) * w])

            for s in range(STAGES):
                stage = spool.tile([NNZ, MM_PER_STAGE * 512], f32)
                for jj in range(MM_PER_STAGE):
                    j = s * MM_PER_STAGE + jj
                    ps = ppool.tile([NNZ, 512], f32)
                    nc.tensor.matmul(
                        ps[:],
                        lhsT=g_t[:],
                        rhs=xt_t[:, j * 512:(j + 1) * 512],
                        start=True,
                        stop=True,
                    )
                    dst = stage[:, jj * 512:(jj + 1) * 512]
                    if jj % 2 == 0:
                        nc.vector.tensor_copy(dst, ps[:])
                    else:
                        nc.scalar.copy(dst, ps[:])
                w = MM_PER_STAGE * 512
                nc.sync.dma_start(out[:, s * w:(s + 1) * w], stage[:])

    nc.compile()
    _CACHE["nc"] = nc
    return nc


def _pack_xt(om, d1, d2, v):
    """Per-core X^T (12, BC) bf16: rows [hi(4); mid(4); lo(4)] of the
    exact 3-term bf16 split of [Omega, d1, d2, V], batch along columns."""
    xt = np.stack([om, d1, d2, v], axis=0)  # (4, BC) f32
    bf = ml_dtypes.bfloat16
    hi = xt.astype(bf)
    r1 = xt - hi.astype(np.float32)
    mid = r1.astype(bf)
    lo = (r1 - mid.astype(np.float32)).astype(bf)
    return np.vstack([hi, mid, lo])  # (12, BC) bf16


def kernel(Omega, Delta, delta_doppler_1, delta_doppler_2, delta_phase,
           V_vdW):
    from concourse.bass_utils import run_bass_kernel_spmd

    nc = _build_module()

    Omega = np.ascontiguousarray(Omega, dtype=np.float32)
    V_vdW = np.ascontiguousarray(V_vdW, dtype=np.float32)
    d1 = (Delta + delta_doppler_1 + delta_phase).astype(np.float32)
    d2 = (Delta + delta_doppler_2 + delta_phase).astype(np.float32)

    in_maps = []
    for c in range(NCORES):
        sl = slice(c * BC, (c + 1) * BC)
        in_maps.append({
            "xt": _pack_xt(Omega[sl], d1[sl], d2[sl], V_vdW[sl]),
            "gmat": G12,
        })

    res = run_bass_kernel_spmd(nc, in_maps, core_ids=list(range(NCORES)))

    out = np.zeros((B, SUP * SUP), dtype=np.complex128)
    out.real[...] = DECAY_REAL.reshape(1, SUP * SUP)
    for c in range(NCORES):
        sl = slice(c * BC, (c + 1) * BC)
        out[sl, NZ_COLS] += 1j * res.results[c]["out"].T.astype(np.float64)
    return out.reshape(B, SUP, SUP)
